# revision 1
# baseline (speedup 1.0000x reference)
"""Trainium2 Bass kernel for nn_NeuronGraph_43336220017086.

Reference semantics:
    h_prev = concat(obs, current[N_IN:])            # [N]
    pre    = W @ h_prev + bias                      # [N]
    pre[rec_dst] += rec_w * history[rec_src, rec_delay]
    return tanh(pre)[-N_OUT:]                       # [N_OUT]

Only the last N_OUT=32 rows of `pre` reach the output, so the kernel
computes exactly those rows.  Work is sharded across 8 NeuronCores:
core c owns output rows [N-32+4c, N-32+4c+4).

Per core:
  - W rows are reshaped [4,8192] -> [128,256] (partition = (row, k-chunk))
    and dotted against a matching replicated h layout with one DVE
    scalar_tensor_tensor (accum_out gives per-partition partial dots).
  - A [128,4] 0/1 reduction matrix contracts the 32 partials per row on
    the tensor engine into PSUM.
  - Recurrent edges with dst in the core's range are gathered from
    history via indirect DMA (flat index src*D+delay) and scattered into
    the same PSUM accumulation through a [128,4] weight matrix matmul.
  - ACT applies tanh(pre + bias) and the 4 results are DMA'd out.
"""

import sys

for _p in ("/opt/trn_rl_repo", "/root/.axon_site/_ro/trn_rl_repo"):
    if _p not in sys.path:
        sys.path.insert(0, _p)

import numpy as np

import concourse.bacc as bacc
import concourse.bass as bass
import concourse.mybir as mybir
from concourse.bass_utils import run_bass_kernel_spmd
from concourse.tile import TileContext

N = 8192
N_IN = 64
N_OUT = 32
D = 4
N_CORES = 8
R_PER_CORE = N_OUT // N_CORES      # 4 output rows per core
KC = 128 // R_PER_CORE             # 32 k-chunks per row
CHUNK = N // KC                    # 256 elements per chunk
CAP_E = 256                       # padded per-core edge capacity
NCOL = CAP_E // 128                # gather columns

_F32 = mybir.dt.float32
_I32 = mybir.dt.int32

_NC = None


def _build_nc():
    nc = bacc.Bacc(
        "TRN2", target_bir_lowering=False, debug=False, num_devices=N_CORES
    )
    wsl = nc.dram_tensor("wsl", [128, CHUNK], _F32, kind="ExternalInput")
    hrep = nc.dram_tensor("hrep", [128, CHUNK], _F32, kind="ExternalInput")
    rmat = nc.dram_tensor("rmat", [128, R_PER_CORE], _F32, kind="ExternalInput")
    smat = nc.dram_tensor("smat", [CAP_E, R_PER_CORE], _F32, kind="ExternalInput")
    eidx = nc.dram_tensor("eidx", [128, NCOL], _I32, kind="ExternalInput")
    hist = nc.dram_tensor("hist", [N * D, 1], _F32, kind="ExternalInput")
    bias4 = nc.dram_tensor("bias4", [R_PER_CORE, 1], _F32, kind="ExternalInput")
    out = nc.dram_tensor("out", [R_PER_CORE, 1], _F32, kind="ExternalOutput")

    with TileContext(nc) as tc:
        with (
            tc.tile_pool(name="sbuf", bufs=1) as pool,
            tc.tile_pool(name="psum", bufs=1, space="PSUM") as pp,
        ):
            w_t = pool.tile([128, CHUNK], _F32)
            nc.sync.dma_start(out=w_t[:, :], in_=wsl[:, :])
            h_t = pool.tile([128, CHUNK], _F32)
            nc.sync.dma_start(out=h_t[:, :], in_=hrep[:, :])
            r_t = pool.tile([128, R_PER_CORE], _F32)
            nc.sync.dma_start(out=r_t[:, :], in_=rmat[:, :])
            s_t = pool.tile([128, NCOL * R_PER_CORE], _F32)
            for c in range(NCOL):
                nc.sync.dma_start(
                    out=s_t[:, c * R_PER_CORE : (c + 1) * R_PER_CORE],
                    in_=smat[c * 128 : (c + 1) * 128, :],
                )
            e_t = pool.tile([128, NCOL], _I32)
            nc.sync.dma_start(out=e_t[:, :], in_=eidx[:, :])
            b_t = pool.tile([R_PER_CORE, 1], _F32)
            nc.sync.dma_start(out=b_t[:, :], in_=bias4[:, :])

            g_t = pool.tile([128, NCOL], _F32)
            for c in range(NCOL):
                nc.gpsimd.indirect_dma_start(
                    out=g_t[:, c : c + 1],
                    out_offset=None,
                    in_=hist[:, :],
                    in_offset=bass.IndirectOffsetOnAxis(
                        ap=e_t[:, c : c + 1], axis=0
                    ),
                )

            prod = pool.tile([128, CHUNK], _F32)
            part = pool.tile([128, 1], _F32)
            nc.vector.scalar_tensor_tensor(
                out=prod[:, :],
                in0=w_t[:, :],
                scalar=1.0,
                in1=h_t[:, :],
                op0=mybir.AluOpType.mult,
                op1=mybir.AluOpType.mult,
                accum_out=part[:, :],
            )

            ps = pp.tile([R_PER_CORE, 1], _F32)
            nc.tensor.matmul(
                out=ps[:, :], lhsT=r_t[:, :], rhs=part[:, :], start=True, stop=False
            )
            for c in range(NCOL):
                nc.tensor.matmul(
                    out=ps[:, :],
                    lhsT=s_t[:, c * R_PER_CORE : (c + 1) * R_PER_CORE],
                    rhs=g_t[:, c : c + 1],
                    start=False,
                    stop=(c == NCOL - 1),
                )

            o_t = pool.tile([R_PER_CORE, 1], _F32)
            nc.scalar.activation(
                o_t[:, :],
                ps[:, :],
                mybir.ActivationFunctionType.Tanh,
                bias=b_t[:, :],
                scale=1.0,
            )
            nc.sync.dma_start(out=out[:, :], in_=o_t[:, :])

    nc.compile()
    return nc


def _get_nc():
    global _NC
    if _NC is None:
        _NC = _build_nc()
    return _NC


def _prep_in_maps(obs, W, bias, current, history, rec_w, rec_src, rec_dst, rec_delay):
    obs = np.asarray(obs, np.float32)
    W = np.asarray(W, np.float32)
    bias = np.asarray(bias, np.float32)
    current = np.asarray(current, np.float32)
    history = np.ascontiguousarray(np.asarray(history, np.float32))
    rec_w = np.asarray(rec_w, np.float32)
    rec_src = np.asarray(rec_src)
    rec_dst = np.asarray(rec_dst)
    rec_delay = np.asarray(rec_delay)

    h = np.concatenate([obs, current[N_IN:]]).astype(np.float32)
    histf = history.reshape(N * D, 1)

    in_maps = []
    for core in range(N_CORES):
        r0 = N - N_OUT + R_PER_CORE * core
        wsl = np.ascontiguousarray(
            W[r0 : r0 + R_PER_CORE].reshape(R_PER_CORE, KC, CHUNK).reshape(128, CHUNK)
        )
        hrep = np.ascontiguousarray(np.tile(h.reshape(KC, CHUNK), (R_PER_CORE, 1)))
        rmat = np.zeros((128, R_PER_CORE), np.float32)
        rmat[np.arange(128), np.arange(128) // KC] = 1.0

        sel = (rec_dst >= r0) & (rec_dst < r0 + R_PER_CORE)
        es = rec_src[sel].astype(np.int64)
        ed = rec_dst[sel].astype(np.int64)
        ew = rec_w[sel]
        edl = rec_delay[sel].astype(np.int64)
        ne = int(es.shape[0])
        if ne > CAP_E:
            # Overflow beyond the padded capacity (astronomically unlikely
            # for this problem size): merge duplicate (flat-idx, dst) pairs.
            flat = es * D + edl
            key = flat * N_OUT + (ed - r0)
            uk, inv = np.unique(key, return_inverse=True)
            acc = np.zeros(uk.shape[0], np.float32)
            np.add.at(acc, inv, ew)
            flat = (uk // N_OUT).astype(np.int64)
            edloc = (uk % N_OUT).astype(np.int64)
            ew = acc
            ne = uk.shape[0]
            assert ne <= CAP_E, f"edge capacity exceeded: {ne}"
        else:
            flat = es * D + edl
            edloc = ed - r0

        eidx = np.zeros((CAP_E,), np.int32)
        eidx[:ne] = flat.astype(np.int32)
        smat = np.zeros((CAP_E, R_PER_CORE), np.float32)
        smat[np.arange(ne), edloc] = ew
        eidx2 = np.ascontiguousarray(eidx.reshape(NCOL, 128).T)

        in_maps.append(
            {
                "wsl": wsl,
                "hrep": hrep,
                "rmat": rmat,
                "smat": smat,
                "eidx": eidx2,
                "hist": histf,
                "bias4": np.ascontiguousarray(
                    bias[r0 : r0 + R_PER_CORE].reshape(R_PER_CORE, 1)
                ),
            }
        )
    return in_maps


def _run(in_maps, trace=False, **kw):
    nc = _get_nc()
    res = run_bass_kernel_spmd(
        nc, in_maps, core_ids=list(range(N_CORES)), trace=trace, **kw
    )
    outs = [res.results[c]["out"].reshape(R_PER_CORE) for c in range(N_CORES)]
    return np.concatenate(outs).astype(np.float32), res


def kernel(**inputs):
    in_maps = _prep_in_maps(**inputs)
    out, _ = _run(in_maps, trace=False)
    return out


# revision 2
# speedup vs baseline: 1.3211x; 1.3211x over previous
"""Trainium2 Bass kernel for nn_NeuronGraph_43336220017086.

Reference semantics:
    h_prev = concat(obs, current[N_IN:])            # [N]
    pre    = W @ h_prev + bias                      # [N]
    pre[rec_dst] += rec_w * history[rec_src, rec_delay]
    return tanh(pre)[-N_OUT:]                       # [N_OUT]

Only the last N_OUT=32 rows of `pre` reach the output, so the kernel
computes exactly those rows.  Work is sharded across 8 NeuronCores:
core c owns output rows [N-32+4c, N-32+4c+4).

Per core:
  - W rows are reshaped [4,8192] -> [128,256] (partition = (row, k-chunk))
    and dotted against a matching replicated h layout with one DVE
    scalar_tensor_tensor (accum_out gives per-partition partial dots).
  - A [128,4] 0/1 reduction matrix contracts the 32 partials per row on
    the tensor engine into PSUM.
  - Recurrent edges with dst in the core's range are gathered from
    history via indirect DMA (flat index src*D+delay) and scattered into
    the same PSUM accumulation through a [128,4] weight matrix matmul.
  - ACT applies tanh(pre + bias) and the 4 results are DMA'd out.

All small per-core tensors (reduction matrix, edge weights, gather
indices, bias) ride in one packed [128, 10] f32 DMA (indices bitcast);
the W slice and replicated h ride in one packed [128, 512] DMA.
"""

import sys

for _p in ("/opt/trn_rl_repo", "/root/.axon_site/_ro/trn_rl_repo"):
    if _p not in sys.path:
        sys.path.insert(0, _p)

import numpy as np

import concourse.bacc as bacc
import concourse.bass as bass
import concourse.mybir as mybir
from concourse.bass_utils import run_bass_kernel_spmd
from concourse.tile import TileContext

N = 8192
N_IN = 64
N_OUT = 32
D = 4
N_CORES = 8
R_PER_CORE = N_OUT // N_CORES      # 4 output rows per core
KC = 128 // R_PER_CORE             # 32 k-chunks per row
CHUNK = N // KC                    # 256 elements per chunk
CAP_E = 128                        # padded per-core edge capacity
NSMALL = 10                        # rmat(4) + smat(4) + eidx(1) + bias(1)

_F32 = mybir.dt.float32
_I32 = mybir.dt.int32

_NC = None


def _build_nc():
    nc = bacc.Bacc(
        "TRN2", target_bir_lowering=False, debug=False, num_devices=N_CORES
    )
    small = nc.dram_tensor("small", [128, NSMALL], _F32, kind="ExternalInput")
    big = nc.dram_tensor("big", [128, 2 * CHUNK], _F32, kind="ExternalInput")
    hist = nc.dram_tensor("hist", [N * D, 1], _F32, kind="ExternalInput")
    out = nc.dram_tensor("out", [R_PER_CORE, 1], _F32, kind="ExternalOutput")

    with TileContext(nc) as tc:
        with (
            tc.tile_pool(name="sbuf", bufs=1) as pool,
            tc.tile_pool(name="psum", bufs=1, space="PSUM") as pp,
        ):
            sm_t = pool.tile([128, NSMALL], _F32)
            nc.sync.dma_start(out=sm_t[:, :], in_=small[:, :])
            r_t = sm_t[:, 0:R_PER_CORE]
            s_t = sm_t[:, R_PER_CORE : 2 * R_PER_CORE]
            e_t = sm_t[:, 2 * R_PER_CORE : 2 * R_PER_CORE + 1].bitcast(_I32)
            b_t = sm_t[0:R_PER_CORE, 2 * R_PER_CORE + 1 : 2 * R_PER_CORE + 2]

            g_t = pool.tile([128, 1], _F32)
            nc.gpsimd.indirect_dma_start(
                out=g_t[:, :],
                out_offset=None,
                in_=hist[:, :],
                in_offset=bass.IndirectOffsetOnAxis(ap=e_t, axis=0),
            )

            w_t = pool.tile([128, 2 * CHUNK], _F32)
            nc.sync.dma_start(out=w_t[:, :], in_=big[:, :])

            prod = pool.tile([128, CHUNK], _F32)
            part = pool.tile([128, 1], _F32)
            nc.vector.scalar_tensor_tensor(
                out=prod[:, :],
                in0=w_t[:, 0:CHUNK],
                scalar=1.0,
                in1=w_t[:, CHUNK : 2 * CHUNK],
                op0=mybir.AluOpType.mult,
                op1=mybir.AluOpType.mult,
                accum_out=part[:, :],
            )

            ps = pp.tile([R_PER_CORE, 1], _F32)
            nc.tensor.matmul(
                out=ps[:, :], lhsT=r_t, rhs=part[:, :], start=True, stop=False
            )
            nc.tensor.matmul(
                out=ps[:, :], lhsT=s_t, rhs=g_t[:, :], start=False, stop=True
            )

            o_t = pool.tile([R_PER_CORE, 1], _F32)
            nc.scalar.activation(
                o_t[:, :],
                ps[:, :],
                mybir.ActivationFunctionType.Tanh,
                bias=b_t,
                scale=1.0,
            )
            nc.sync.dma_start(out=out[:, :], in_=o_t[:, :])

    nc.compile()
    return nc


def _get_nc():
    global _NC
    if _NC is None:
        _NC = _build_nc()
    return _NC


def _prep_in_maps(obs, W, bias, current, history, rec_w, rec_src, rec_dst, rec_delay):
    obs = np.asarray(obs, np.float32)
    W = np.asarray(W, np.float32)
    bias = np.asarray(bias, np.float32)
    current = np.asarray(current, np.float32)
    history = np.ascontiguousarray(np.asarray(history, np.float32))
    rec_w = np.asarray(rec_w, np.float32)
    rec_src = np.asarray(rec_src)
    rec_dst = np.asarray(rec_dst)
    rec_delay = np.asarray(rec_delay)

    h = np.concatenate([obs, current[N_IN:]]).astype(np.float32)
    histf = history.reshape(N * D, 1)
    hrep = np.tile(h.reshape(KC, CHUNK), (R_PER_CORE, 1))

    in_maps = []
    for core in range(N_CORES):
        r0 = N - N_OUT + R_PER_CORE * core
        big = np.empty((128, 2 * CHUNK), np.float32)
        big[:, 0:CHUNK] = (
            W[r0 : r0 + R_PER_CORE].reshape(R_PER_CORE, KC, CHUNK).reshape(128, CHUNK)
        )
        big[:, CHUNK:] = hrep

        sel = (rec_dst >= r0) & (rec_dst < r0 + R_PER_CORE)
        es = rec_src[sel].astype(np.int64)
        ed = rec_dst[sel].astype(np.int64)
        ew = rec_w[sel]
        edl = rec_delay[sel].astype(np.int64)
        flat = es * D + edl
        edloc = ed - r0
        ne = int(es.shape[0])
        if ne > CAP_E:
            # Overflow beyond the padded capacity (astronomically unlikely
            # for this problem size): merge duplicate (flat-idx, dst) pairs.
            key = flat * N_OUT + edloc
            uk, inv = np.unique(key, return_inverse=True)
            acc = np.zeros(uk.shape[0], np.float32)
            np.add.at(acc, inv, ew)
            flat = (uk // N_OUT).astype(np.int64)
            edloc = (uk % N_OUT).astype(np.int64)
            ew = acc
            ne = uk.shape[0]
            assert ne <= CAP_E, f"edge capacity exceeded: {ne}"

        small = np.zeros((128, NSMALL), np.float32)
        # reduction matrix: partition p reduces into row p // KC
        small[np.arange(128), np.arange(128) // KC] = 1.0
        # edge weight scatter matrix
        small[np.arange(ne), R_PER_CORE + edloc] = ew
        # gather indices (int32 bits carried in the f32 pack)
        eidx = np.zeros((128,), np.int32)
        eidx[:ne] = flat.astype(np.int32)
        small[:, 2 * R_PER_CORE] = eidx.view(np.float32)
        # bias in partitions [0, R_PER_CORE)
        small[0:R_PER_CORE, 2 * R_PER_CORE + 1] = bias[r0 : r0 + R_PER_CORE]

        in_maps.append({"small": small, "big": big, "hist": histf})
    return in_maps


def _run(in_maps, trace=False, **kw):
    nc = _get_nc()
    res = run_bass_kernel_spmd(
        nc, in_maps, core_ids=list(range(N_CORES)), trace=trace, **kw
    )
    outs = [res.results[c]["out"].reshape(R_PER_CORE) for c in range(N_CORES)]
    return np.concatenate(outs).astype(np.float32), res


def kernel(**inputs):
    in_maps = _prep_in_maps(**inputs)
    out, _ = _run(in_maps, trace=False)
    return out
